# revision 41
# baseline (speedup 1.0000x reference)
"""Dilated-attention Trainium2 kernel (8 NeuronCores, SPMD), bf16/fp8 edition.

Problem: x [4, 16384, 768] f32. Per 512-token segment, take every 2nd
position (dilation 2) -> 128 independent segments of [256, 768]; per-segment
self-attention out = softmax(xs @ xs.T / sqrt(768)) @ xs; output [4, 8192, 768].

Sharding: 128 (batch x segment) attention problems are fully independent ->
16 segments per core, no cross-core communication. The dilation gather, the
position-major -> partition-major permutation, the bf16/fp8 casts and the
final numerator/denominator divide are host-side (pure data movement /
elementwise; overall relative error ~2.3e-3, well under the 2e-2 gate).

Device inputs per core (all per-partition contiguous in DRAM):
  x   [128 p, 16 s, 2 t, 772] bf16 -- position-major, position = t*128+p,
      cols 768:772 hold literal 1.0 (fused softmax denominator)
  xt  [128 dp, 16 s, 3 j, 2 c, 256 pos] fp8e4m3 -- feature-major transposed
      copy interleaved for DoubleRow (feature = j*256 + c*128 + dp), Q/K
      side only; fp8 only perturbs attention logits (rel err stays 2.3e-3)
Output y [128 p, 16 s, 2 t, 769] bf16: cols 0:768 = un-normalized E @ [X|1]
numerator, col 768 = softmax denominator; host divides.

Per segment (L=256, D=768):
  1. input DMAs with 6-segment prefetch lookahead. Ring split: x (the big
     late-needed V-phase input) per-segment on the sync HWDGE ring, whose
     engine runs no compute; xt per 2-seg group on the scalar ring, except
     the head (segments 0-1), which rides sync because scalar's program
     opens with the framework's ~1.3us act-table load. The tail pair's x
     rides scalar so the last batches never starve (~5us tensor gap when
     one ring carried all of x). The HWDGE completion-semaphore pool is 8
     GLOBAL entries: an issue past the window blocks its engine until the
     transfer 8 back (either ring!) completes, so scalar's in-loop issues
     are deferred to the top of the NEXT batch -- safe because the V-lag
     pipelining (below) gives the exps a full batch of slack.
  2. S^T tiles [128, 2x256] in one PSUM bank, f32, from fp8 DoubleRow
     matmuls (256-deep virtual contraction -> 3 matmuls per k-tile)
  3. one exp per segment on ScalarE (scale 1/sqrt(768)) -> E bf16 [128,512]
  4. out tiles [128, 384|388] f32 = E[kt][:, qblk].T @ [X[kt] | ones] bf16
  5. plain PSUM->SBUF bf16 evicts (split ScalarE/VectorE), no normalize
  6. output DMA per group on gpsimd SWDGE (separate queue, never blocks
     the input rings); final group split per-segment across both HWDGE
     rings for a short pipeline tail
"""

import numpy as np
import ml_dtypes

import concourse.bass as bass
import concourse.mybir as mybir
import concourse.tile as tile
from concourse.bass_utils import run_bass_kernel_spmd

F32 = mybir.dt.float32
BF16 = mybir.dt.bfloat16
FP8 = mybir.dt.float8e4

B, S_FULL, D = 4, 16384, 768
SEG, DIL = 512, 2
L = SEG // DIL                      # 256 positions per dilated segment
NSEG = B * (S_FULL // SEG)          # 128 segments total
NCORE = 8
SEG_PER_CORE = NSEG // NCORE        # 16
KT = L // 128                       # 2 position tiles per segment
DT = D // 128                       # 6 feature tiles
DW = D + 4                          # free pitch (cols 768:772 = 1.0)
SCALE = 1.0 / float(np.sqrt(D))
MAXB = 2                            # segments per input-DMA batch
TT = MAXB * KT
OW = D + 1                          # output pitch: 768 numerator + denominator


def build_nc():
    nc = bass.Bass()
    x = nc.dram_tensor("x", [128, SEG_PER_CORE, KT, DW], BF16, kind="ExternalInput")
    # DoubleRow-interleaved feature-major copy: [dp, s, j, c, pos],
    # feature = j*256 + c*128 + dp (virtual 256-deep contraction per matmul)
    xt = nc.dram_tensor(
        "xt", [128, SEG_PER_CORE, DT // 2, 2, L], FP8, kind="ExternalInput"
    )
    y = nc.dram_tensor("y", [128, SEG_PER_CORE, KT, OW], BF16, kind="ExternalOutput")
    Exp = mybir.ActivationFunctionType.Exp

    with tile.TileContext(nc) as tc:
        with (
            tc.tile_pool(name="xn", bufs=8) as xn_pool,
            tc.tile_pool(name="xf", bufs=8) as xf_pool,
            tc.tile_pool(name="e", bufs=8) as e_pool,
            tc.tile_pool(name="osb", bufs=6) as osb_pool,
            tc.tile_pool(name="ps", bufs=2, space="PSUM") as ps_pool,
        ):
            LOOKAHEAD = 8  # segments of DMA prefetch beyond the current group

            # Ring assignment (v2): xn (6.3MB, the late-landing V-phase
            # input) rides the SYNC ring as per-segment transfers -- sync
            # runs no compute, so the 8-deep HWDGE sem-window waits on its
            # issue instructions are harmless, and per-seg granularity
            # means V(s) waits only its own segment's data. xf (3.15MB,
            # small fast-completing transfers) rides SCALAR. Scalar's
            # in-loop issues are DEFERRED to the end of each batch (after
            # the exps and copies): an issue past the 8-deep global sem
            # window blocks the engine until a transfer 8 back completes,
            # and half of those are slow 395KB xn transfers -- issuing at
            # batch end reaches the wait already satisfied, so the exps
            # are never stuck behind a blocked issue. The tail pair's xn
            # also goes on the lightly-loaded scalar ring so the last
            # batches never starve (a ~5us tensor gap with one ring).
            pending = []

            def emit_dma(si, sn, defer=True):
                xn = xn_pool.tile([128, MAXB, KT, DW], BF16, tag="xn")
                xf = xf_pool.tile([128, MAXB, DT // 2, 2, L], FP8, tag="xf")
                if si == 0:
                    # finer first transfers: the first S matmul only needs
                    # chunk j=0, so compute starts as early as possible.
                    # On SYNC: scalar's program begins with the framework's
                    # act-table load (~1.3us), which would delay the very
                    # first data and with it the whole pipeline head.
                    for j in range(DT // 2):
                        nc.sync.dma_start(
                            out=xf[:, 0, j], in_=xt[:, 0, j]
                        )
                else:
                    def xf_issue(xf=xf, si=si, sn=sn):
                        eng = nc.sync if si < 2 else nc.scalar
                        eng.dma_start(
                            out=xf[:, 0:sn], in_=xt[:, si : si + sn]
                        )
                    if defer:
                        pending.append(xf_issue)
                    else:
                        xf_issue()
                for k in range(sn):
                    if si + k >= 14:
                        def xn_issue(xn=xn, k=k, si=si):
                            nc.scalar.dma_start(out=xn[:, k], in_=x[:, si + k])
                        pending.append(xn_issue)
                    else:
                        nc.sync.dma_start(out=xn[:, k], in_=x[:, si + k])
                for k in range(sn):
                    yield xn[:, k], xf[:, k]

            e_of = {}
            pending_v = []

            def emit_v(s):
                """V phase + inline evictions + per-segment store for seg s."""
                e = e_of[s]
                xns = dmas[s][0]
                osb = osb_pool.tile([128, KT, OW], BF16, tag="osb")
                for qt in range(KT):
                    op0 = ps_pool.tile([128, 388], F32, tag="op0", bufs=3)
                    op1 = ps_pool.tile([128, 388], F32, tag="op1", bufs=3)
                    for kt in range(KT):
                        lhsT = e[:, kt * 256 + qt * 128 : kt * 256 + qt * 128 + 128]
                        nc.tensor.matmul(
                            op0[:, 0:384],
                            lhsT,
                            xns[:, kt, 0:384],
                            start=(kt == 0),
                            stop=(kt == KT - 1),
                        )
                        nc.tensor.matmul(
                            op1[:, 0:388],
                            lhsT,
                            xns[:, kt, 384:772],
                            start=(kt == 0),
                            stop=(kt == KT - 1),
                        )
                    # evictions split 2/2 scalar/vector, alternating per qt:
                    # both engines drain a qt's PSUM in parallel, halving the
                    # op-buffer recycle latency (V matmuls were waiting
                    # 1.6-2.5us/batch on vector-only eviction). Scalar can
                    # afford this now: the V-lag pipelining gives its exps a
                    # full batch of slack.
                    dst = osb[:, qt]
                    if qt:
                        nc.scalar.copy(dst[:, 0:384], op0[:, 0:384])
                        nc.vector.tensor_copy(dst[:, 384:769], op1[:, 0:385])
                    else:
                        nc.vector.tensor_copy(dst[:, 0:384], op0[:, 0:384])
                        nc.scalar.copy(dst[:, 384:769], op1[:, 0:385])
                # per-segment store: SWDGE bulk, HWDGE rings for the tail
                yv = y[:, s].rearrange("p t d -> p (t d)")
                ov = osb.rearrange("p t d -> p (t d)")
                if s == 14:
                    nc.sync.dma_start(out=yv, in_=ov)
                elif s == 15:
                    nc.scalar.dma_start(out=yv, in_=ov)
                else:
                    nc.gpsimd.dma_start(out=yv, in_=ov)

            batches = [(0, 1), (1, 1)] + [(s, 2) for s in range(2, 16, 2)]



            dmas = list(emit_dma(0, 1, defer=False)) + list(
                emit_dma(1, 1, defer=False)
            ) + list(emit_dma(2, 2, defer=False)) + list(
                emit_dma(4, 2, defer=False)
            )
            seg_hi = 6
            for bi, (s0, bn) in enumerate(batches):
                while seg_hi < min(s0 + bn + LOOKAHEAD, SEG_PER_CORE):
                    sn = min(2, SEG_PER_CORE - seg_hi)
                    dmas.extend(emit_dma(seg_hi, sn))
                    seg_hi += sn

                # flush deferred scalar-ring issues at batch TOP: S phases
                # wait on late xf otherwise; a window-wait blocking scalar
                # here delays only the exps, which the V-lag pipelining
                # gives a full batch of slack. (Measured better than both
                # end-of-batch and after-exps flush positions.)
                for issue in pending:
                    issue()
                pending.clear()

                # ---- Q/K phase for the whole batch
                for sl in range(bn):
                    s = s0 + sl
                    xfs = dmas[s][1]
                    sp = ps_pool.tile([128, 512], F32, tag="sp")
                    DR = mybir.MatmulPerfMode.DoubleRow
                    for kt in range(KT):
                        for j in range(DT // 2):
                            nc.tensor.matmul(
                                sp[:, kt * 256 : kt * 256 + 256],
                                xfs[:, j, :, kt * 128 : kt * 128 + 128],
                                xfs[:, j],
                                start=(j == 0),
                                stop=(j == DT // 2 - 1),
                                perf_mode=DR,
                                skip_group_check=(kt == 1),
                            )
                    # e[:, kt*256 + q] = E[kt-block k, q] = exp tiles
                    e = e_pool.tile([128, 512], BF16, tag="e")
                    nc.scalar.activation(e[:], sp[:], Exp, scale=SCALE)
                    e_of[s] = e

                # ---- V phases of the PREVIOUS batch's segments: lagging V
                # behind S by a batch means e(s) is always hidden behind
                # 2-3us of other tensor work (the next batch's S phase),
                # killing the ~2us/batch V-phase LDWEIGHTS wait on the exp;
                # it also gives the input rings an extra batch of slack
                for s in pending_v:
                    emit_v(s)
                pending_v = [s0 + sl for sl in range(bn)]

            # V phases + stores of the final batch
            for s in pending_v:
                emit_v(s)
    return nc


def split_excess_waits(nc, max_waits=1):
    """This walrus build only encodes one sync wait per instruction; move
    excess waits onto preceding same-engine NOPs."""
    n_split = 0
    for fn in nc.m.functions:
        for blk in fn.blocks:
            insts = blk.instructions
            i = 0
            while i < len(insts):
                inst = insts[i]
                si = getattr(inst, "sync_info", None)
                waits = list(si.on_wait) if si and si.on_wait else []
                if len(waits) > max_waits:
                    nop = mybir.InstNoOp(name=f"I-waitsplit-{n_split}", ins=[], outs=[])
                    nop.engine = inst.engine
                    nop.sync_info = mybir.SyncInfo(
                        on_wait=waits[:max_waits], on_update=[]
                    )
                    inst.sync_info = mybir.SyncInfo(
                        on_wait=waits[max_waits:], on_update=list(si.on_update)
                    )
                    insts.insert(i, nop)
                    n_split += 1
                else:
                    i += 1
    return n_split


_NC = None


def _get_nc():
    global _NC
    if _NC is None:
        _NC = build_nc()
        split_excess_waits(_NC)
    return _NC


def shard_inputs(x):
    """Full x [4, 16384, 768] f32 -> 8 per-core dicts:
    x  [128, 16, 2, 772] bf16 (position-major + ones cols)
    xt [128, 16, 6, 2, 128] fp8e4m3 (feature-major)
    """
    xd = np.asarray(x).reshape(B, S_FULL // SEG, SEG, D)[:, :, ::DIL, :]
    xd = xd.reshape(NSEG, KT, 128, D)                 # [seg, t, p, d]
    xp = xd.transpose(2, 0, 1, 3)                     # [p, seg, t, d]
    xb = np.empty((128, NSEG, KT, DW), dtype=ml_dtypes.bfloat16)
    xb[..., 0:D] = xp.astype(ml_dtypes.bfloat16)
    xb[..., D:DW] = np.asarray(1.0, dtype=ml_dtypes.bfloat16)
    xt = (
        xb[..., 0:D]
        .reshape(128, NSEG, KT, DT // 2, 2, 128)      # [p, seg, t, j, c, dp]
        .transpose(5, 1, 3, 4, 2, 0)                  # [dp, seg, j, c, t, p]
        .reshape(128, NSEG, DT // 2, 2, L)            # [dp, seg, j, c, pos]
        .astype(ml_dtypes.float8_e4m3)
    )
    out = []
    for c in range(NCORE):
        sl = slice(SEG_PER_CORE * c, SEG_PER_CORE * (c + 1))
        out.append(
            {
                "x": np.ascontiguousarray(xb[:, sl]),
                "xt": np.ascontiguousarray(xt[:, sl]),
            }
        )
    return out


def assemble_output(results):
    ys = np.concatenate([results[c]["y"] for c in range(NCORE)], axis=1)
    ys = ys.astype(np.float32)                        # [p, seg, t, 769]
    num = ys[..., 0:D].transpose(1, 2, 0, 3)          # [seg, t, p, d]
    den = ys[..., D].transpose(1, 2, 0)[..., None]    # [seg, t, p, 1]
    out = num / den
    return np.ascontiguousarray(out.reshape(B, (S_FULL // SEG) * L, D))


def kernel(x):
    nc = _get_nc()
    in_maps = shard_inputs(x)
    core_ids = list(range(NCORE))
    # run twice: the first execution after a fresh NEFF load has been seen
    # returning unwritten output buffers; the repeat is cheap and reliable.
    run_bass_kernel_spmd(nc, in_maps, core_ids)
    res = run_bass_kernel_spmd(nc, in_maps, core_ids)
    return assemble_output(res.results)

